# revision 1
# baseline (speedup 1.0000x reference)
"""Trainium2 Bass kernel for DMPNN encoder (nn_DMPNNEncoder_61907658604848).

Strategy (data-parallel over molecules, 8 NeuronCores):
  - Each molecule is a 24-atom ring => 48 directed bonds. After a host-side
    bond reordering ("eo order": even bonds in atom order, odd bonds reversed
    within each molecule), the message-passing gather
        m = padded[bibm].sum(1) - h[rev]
    becomes a uniform cyclic shift-down-by-1 within every 24-column block.
  - All activations live transposed [channel, bond-col] so W-matmuls contract
    over channels (PE partitions), BN stats are per-partition free-dim
    reductions (bn_stats/bn_aggr + AllReduce of per-core sums), and the BN
    affine+relu are per-partition tensor_scalar ops.
  - h (fp16) stays SBUF-resident; y and h0 round-trip through DRAM.
"""

import os

os.environ.setdefault("MYCRO_LOCAL_CACHE", "1")

import numpy as np

# ---------------- problem constants (hardcoded per spec) ----------------
M_TOT = 4096
A = 24
H = 300
ATOM_DIM = 133
BOND_DIM = 14
MOL_DIM = 200
DEPTH = 4
EPS = 1e-5
NCORES = 8

SLABS = [(0, 128), (128, 256), (256, 300)]  # output-channel slabs
KCH = [(0, 128), (128, 256), (256, 300)]    # contraction chunks over H

F16 = np.float16


# ---------------- host-side data prep ----------------
def _eo_perm(n_mols):
    """perm[eo_col] = original directed-bond index within the shard."""
    mu = np.arange(n_mols)[:, None]
    v = np.arange(A)[None, :]
    e = (mu * 48 + 2 * v).reshape(-1)                    # e-lane: col mu*24+v
    o = (mu * 48 + 2 * (A - 1 - v) + 1).reshape(-1)      # o-lane col mu*24+w -> v=23-w
    return np.concatenate([e, o])


def _prep_core_inputs(inputs, p, mpc):
    half = mpc * A
    a0 = p * half
    e0 = p * 2 * half
    af = np.asarray(inputs["atom_features"], np.float32)[a0:a0 + half]
    bf = np.asarray(inputs["bond_features"], np.float32)[e0:e0 + 2 * half]
    perm = _eo_perm(mpc)

    af_T = np.ascontiguousarray(af.T, dtype=F16)                      # [133, half]
    af_Tr = np.ascontiguousarray(
        af_T.reshape(ATOM_DIM, mpc, A)[:, :, ::-1].reshape(ATOM_DIM, half))
    bf_T = np.ascontiguousarray(bf[perm].T, dtype=F16)                # [14, 2*half]

    W_i = np.asarray(inputs["W_i"], np.float32)   # [300, 147] = [bond14 | atom133]
    W_m = np.asarray(inputs["W_m"], np.float32)
    W_a = np.asarray(inputs["W_a"], np.float32)   # [300, 433] = [atom133 | mv300]
    Wb = W_i[:, :BOND_DIM]
    Wat = W_i[:, BOND_DIM:]
    # lhsT chunks: rhs K-stacking order is [af 0:128 | af 128:133 | bf 14]
    WiT0 = np.ascontiguousarray(Wat[:, 0:128].T, dtype=F16)           # [128, 300]
    WiT1 = np.ascontiguousarray(
        np.concatenate([Wat[:, 128:133], Wb], axis=1).T, dtype=F16)   # [19, 300]
    WmT = np.ascontiguousarray(W_m.T, dtype=F16)                      # [300, 300]
    WaT = np.ascontiguousarray(W_a.T, dtype=F16)                      # [433, 300]

    d = {
        "af_T": af_T, "af_Tr": af_Tr, "bf_T": bf_T,
        "WiT0": WiT0, "WiT1": WiT1, "WmT": WmT, "WaT": WaT,
    }
    for n in ("g_i", "b_i", "g_m", "b_m", "g_a", "b_a"):
        v = np.asarray(inputs[n], np.float32)
        pk = np.zeros((128, 3), np.float32)
        for s, (s0, s1) in enumerate(SLABS):
            pk[0:s1 - s0, s] = v[s0:s1]
        d[n] = pk
    return d


# ---------------- bass program ----------------
def _build_program(mpc, repeat=1):
    import concourse.bass as bass
    import concourse.bacc as bacc
    import concourse.tile as tile
    import concourse.mybir as mybir

    f16 = mybir.dt.float16
    f32 = mybir.dt.float32
    MUL = mybir.AluOpType.mult
    ADD = mybir.AluOpType.add
    SUB = mybir.AluOpType.subtract

    HALF = mpc * A
    EC = 2 * HALF
    CT = 512 if HALF % 512 == 0 else HALF          # phase-A col-tile
    CC = 1536 if HALF % 1536 == 0 else HALF        # phase-C chunk (24 | CC)
    assert HALF % CT == 0 and HALF % CC == 0 and CC % A == 0
    NTL = HALF // CT                               # phase-A tiles per lane
    NCH = EC // CC                                 # phase-C chunks per slab
    NB = CC // A                                   # 24-blocks per chunk
    WY = EC + 24                                   # y_d padded width (col c -> idx c+1)
    NCHL = HALF // CC                              # phase-C chunks per lane
    TACT = (2 * NTL) * 20 // 48                    # leading tiles with ACT stats
    TACT_R = NTL * 10 // 24
    HEAT_B = 24                                    # PE heater matmuls per phase B
    CNT_G = float(EC * NCORES)                     # global batch size (bonds)

    nc = bacc.Bacc("TRN2", target_bir_lowering=False, debug=False,
                   num_devices=NCORES)

    # ---- I/O ----
    def din(name, shape, dt=f16):
        return nc.dram_tensor(name, shape, dt, kind="ExternalInput").ap()

    af_T = din("af_T", [ATOM_DIM, HALF])
    af_Tr = din("af_Tr", [ATOM_DIM, HALF])
    bf_T = din("bf_T", [BOND_DIM, EC])
    WiT0_d = din("WiT0", [128, H])
    WiT1_d = din("WiT1", [19, H])
    WmT_d = din("WmT", [H, H])
    WaT_d = din("WaT", [433, H])
    gb_d = {n: din(n, [128, 3], f32) for n in ("g_i", "b_i", "g_m", "b_m", "g_a", "b_a")}
    h_mol = nc.dram_tensor("h_mol", [3, 128, mpc], f32, kind="ExternalOutput").ap()

    ctx_stack = []

    with tile.TileContext(nc) as tc:
        import contextlib
        ctx = contextlib.ExitStack()

        res = ctx.enter_context(tc.tile_pool(name="res", bufs=1))
        dram = ctx.enter_context(tc.tile_pool(name="dram", bufs=1, space="DRAM"))
        dramx = ctx.enter_context(tc.tile_pool(name="dramx", bufs=2, space="DRAM"))
        psum = ctx.enter_context(tc.tile_pool(name="psum", bufs=6, space="PSUM"))
        stage = ctx.enter_context(tc.tile_pool(name="stage", bufs=6))
        stream = ctx.enter_context(tc.tile_pool(name="stream", bufs=3))
        chunkp = ctx.enter_context(tc.tile_pool(name="chunkp", bufs=2))
        statsp = ctx.enter_context(tc.tile_pool(name="statsp", bufs=2))
        heatp = ctx.enter_context(tc.tile_pool(name="heatp", bufs=2, space="PSUM"))

        # ---- resident SBUF ----
        h_sb = [[res.tile([128, HALF], f16, tag=f"h{s}{l}", name=f"h{s}{l}")
                 for l in range(2)] for s in range(3)]
        wi0_sb = res.tile([128, H], f16, tag="wi0", name="wi0")
        wi1a_sb = res.tile([5, H], f16, tag="wi1a", name="wi1a")
        wi1b_sb = res.tile([BOND_DIM, H], f16, tag="wi1b", name="wi1b")
        wm_sb = [res.tile([k1 - k0, H], f16, tag=f"wm{i}", name=f"wm{i}")
                 for i, (k0, k1) in enumerate(KCH)]
        # WaT row chunks: [0:128]=af0, [128:133]=af1, then mv chunks offset by 133
        wa_af0 = res.tile([128, H], f16, tag="wa_af0", name="wa_af0")
        wa_af1 = res.tile([5, H], f16, tag="wa_af1", name="wa_af1")
        wa_mv = [res.tile([k1 - k0, H], f16, tag=f"wa_mv{i}", name=f"wa_mv{i}")
                 for i, (k0, k1) in enumerate(KCH)]
        gb_sb = {n: res.tile([128, 3], f32, tag=f"gb_{n}", name=f"gb_{n}") for n in gb_d}
        eps_sb = res.tile([128, 1], f32, tag="eps", name="eps")
        w_heat = res.tile([64, 64], f32, tag="w_heat", name="w_heat")
        sc_sb = res.tile([128, 3], f32, tag="sc", name="sc")
        bi_sb = res.tile([128, 3], f32, tag="bi", name="bi")

        nc.vector.memset(eps_sb, EPS)
        nc.vector.memset(w_heat, 0.0)
        nc.sync.dma_start(out=wi0_sb, in_=WiT0_d)
        nc.sync.dma_start(out=wi1a_sb, in_=WiT1_d[0:5])
        nc.sync.dma_start(out=wi1b_sb, in_=WiT1_d[5:19])
        for i, (k0, k1) in enumerate(KCH):
            nc.sync.dma_start(out=wm_sb[i], in_=WmT_d[k0:k1])
            nc.sync.dma_start(out=wa_mv[i], in_=WaT_d[133 + k0:133 + k1])
        nc.sync.dma_start(out=wa_af0, in_=WaT_d[0:128])
        nc.sync.dma_start(out=wa_af1, in_=WaT_d[128:133])
        for n, dap in gb_d.items():
            nc.sync.dma_start(out=gb_sb[n], in_=dap)

        # ---- persistent DRAM scratch ----
        # y is double-buffered by pipeline stage parity: stage k writes
        # y_d2[k%2] while phase C of stage k-1 still reads y_d2[(k-1)%2],
        # breaking the WAR chain that would serialize consecutive stages.
        y_d2 = [[dram.tile([128, WY], f16, tag=f"y{p}{s}", name=f"y{p}{s}")
                 for s in range(3)] for p in range(2)]
        h0_d = [dram.tile([128, EC], f16, tag=f"h0{s}", name=f"h0{s}") for s in range(3)]


        # -------- phase A: matmul -> psum -> stats + f16 evac -> y_d --------
        # The first `tact` tiles keep their stats entirely on ACT (evac copy
        # with accum_out for the sum + a Square pass for the sumsq) so the PE
        # and ACT can run them while DVE is still finishing the previous
        # phase C. The remaining tiles use one DVE bn_stats op per psum tile.
        def phase_a(y_d, nt, rhs_chunks_fn, stats6, sum_p, ssq_p, tact):
            for t in range(nt):
                chunks = rhs_chunks_fn(t)
                for s, (s0, s1) in enumerate(SLABS):
                    ms = s1 - s0
                    ps = psum.tile([128, CT], mybir.dt.float32, tag="ps", name="ps")
                    for i, (lh, rh) in enumerate(chunks):
                        nc.tensor.matmul(ps[0:ms], lh[:, s0:s1], rh,
                                         start=(i == 0), stop=(i == len(chunks) - 1))
                    st = stage.tile([128, CT], f16, tag="st", name="st")
                    if t < tact:
                        nc.scalar.activation(out=st[0:ms], in_=ps[0:ms],
                                             func=mybir.ActivationFunctionType.Copy,
                                             accum_out=sum_p[0:ms, s, t:t + 1])
                        sq = stage.tile([128, CT], f16, tag="sq", name="sq")
                        nc.scalar.activation(out=sq[0:ms], in_=ps[0:ms],
                                             func=mybir.ActivationFunctionType.Square,
                                             accum_out=ssq_p[0:ms, s, t:t + 1])
                    else:
                        nc.scalar.copy(out=st[0:ms], in_=ps[0:ms])
                        nc.vector.bn_stats(out=stats6[0:ms, s, t - tact], in_=ps[0:ms])
                    nc.sync.dma_start(out=y_d[s][0:ms, 1 + t * CT:1 + (t + 1) * CT],
                                      in_=st[0:ms])

        # -------- phase B: stats -> allreduce -> scale/bias --------
        def phase_b(stats6, sum_p, ssq_p, nt, tact, gname, bname, cnt_g=None):
            tdve = nt - tact
            cnt_d = float(tdve * CT)
            cnt_g = CNT_G if cnt_g is None else cnt_g
            pack = statsp.tile([128, 6], f32, tag="pack", name="pack")
            micro = statsp.tile([128, 24], f32, tag="micro", name="micro")
            nc.vector.memset(pack, 0.0)
            for s, (s0, s1) in enumerate(SLABS):
                ms = s1 - s0
                mv = micro[0:ms, 8 * s:8 * s + 2]
                nc.vector.bn_aggr(out=mv, in_=stats6[0:ms, s, 0:tdve])
                mean = micro[0:ms, 8 * s:8 * s + 1]
                var = micro[0:ms, 8 * s + 1:8 * s + 2]
                sq = micro[0:ms, 8 * s + 2:8 * s + 3]
                ss = micro[0:ms, 8 * s + 3:8 * s + 4]
                # DVE part: sum = mean*cnt_d ; sumsq = (var + mean^2)*cnt_d
                nc.vector.tensor_tensor(sq, mean, mean, MUL)
                nc.vector.tensor_tensor(ss, var, sq, ADD)
                nc.vector.tensor_scalar_mul(ss, ss, cnt_d)
                nc.vector.tensor_scalar_mul(mean, mean, cnt_d)
                if tact > 0:
                    nc.vector.tensor_reduce(out=sq, in_=ssq_p[0:ms, s],
                                            axis=mybir.AxisListType.X, op=ADD)
                    nc.vector.tensor_tensor(pack[0:ms, 2 * s + 1:2 * s + 2],
                                            ss, sq, ADD)
                    nc.vector.tensor_reduce(out=sq, in_=sum_p[0:ms, s],
                                            axis=mybir.AxisListType.X, op=ADD)
                    nc.vector.tensor_tensor(pack[0:ms, 2 * s:2 * s + 1],
                                            mean, sq, ADD)
                else:
                    nc.vector.tensor_copy(pack[0:ms, 2 * s + 1:2 * s + 2], ss)
                    nc.vector.tensor_copy(pack[0:ms, 2 * s:2 * s + 1], mean)
            din_t = dramx.tile([128, 6], f32, tag="cc_in", name="cc_in")
            dout_t = dramx.tile([128, 6], f32, tag="cc_out", name="cc_out")
            nc.sync.dma_start(out=din_t, in_=pack)
            nc.gpsimd.collective_compute(
                "AllReduce", ADD, replica_groups=[list(range(NCORES))],
                ins=[din_t[:].opt()], outs=[dout_t[:].opt()])
            # PE heater: ~90 tiny matmuls bridge the collective's PE-idle gap
            # so HAM stays at 8/8 (they sit in PE program order between the
            # phase-A and next-phase matmuls; rhs dep is the pack tile).
            for _h in range(HEAT_B):
                hps = heatp.tile([64, 64], mybir.dt.float32, tag="hps", name="hps")
                nc.tensor.matmul(hps[0:64, 0:6], w_heat,
                                 pack[0:64, 0:6], start=True, stop=True)
            allst = statsp.tile([128, 6], f32, tag="allst", name="allst")
            nc.sync.dma_start(out=allst, in_=dout_t)
            for s, (s0, s1) in enumerate(SLABS):
                ms = s1 - s0
                mu = micro[0:ms, 8 * s + 4:8 * s + 5]
                ey2 = micro[0:ms, 8 * s + 5:8 * s + 6]
                var = micro[0:ms, 8 * s + 6:8 * s + 7]
                tmp = micro[0:ms, 8 * s + 7:8 * s + 8]
                nc.vector.tensor_scalar_mul(mu, allst[0:ms, 2 * s:2 * s + 1], 1.0 / cnt_g)
                nc.vector.tensor_scalar_mul(ey2, allst[0:ms, 2 * s + 1:2 * s + 2], 1.0 / cnt_g)
                nc.vector.tensor_tensor(tmp, mu, mu, MUL)
                nc.vector.tensor_tensor(var, ey2, tmp, SUB)
                # rstd = 1/sqrt(var+eps):  ACT Sqrt (bias=eps) then DVE reciprocal
                nc.scalar.activation(out=var, in_=var,
                                     func=mybir.ActivationFunctionType.Sqrt,
                                     bias=eps_sb[0:ms, 0:1], scale=1.0)
                nc.vector.reciprocal(out=var, in_=var)
                nc.vector.tensor_tensor(sc_sb[0:ms, s:s + 1],
                                        gb_sb[gname][0:ms, s:s + 1], var, MUL)
                nc.vector.tensor_tensor(tmp, mu, sc_sb[0:ms, s:s + 1], MUL)
                nc.vector.tensor_tensor(bi_sb[0:ms, s:s + 1],
                                        gb_sb[bname][0:ms, s:s + 1], tmp, SUB)

        # -------- phase C variants (a is a global bond col; lane-local col al)
        def phase_c_l0(y_d, s, ms, lane, al):
            a = lane * HALF + al
            x0 = chunkp.tile([128, CC], f16, tag="X0", name="X0")
            tt_ = chunkp.tile([128, CC], f16, tag="T", name="T")
            nc.scalar.dma_start(out=x0[0:ms], in_=y_d[s][0:ms, 1 + a:1 + a + CC])
            nc.vector.tensor_scalar(out=tt_[0:ms], in0=x0[0:ms],
                                    scalar1=sc_sb[0:ms, s:s + 1],
                                    scalar2=bi_sb[0:ms, s:s + 1],
                                    op0=MUL, op1=ADD)
            nc.vector.tensor_scalar_max(h_sb[s][lane][0:ms, al:al + CC], tt_[0:ms], 0.0)
            nc.sync.dma_start(out=h0_d[s][0:ms, a:a + CC],
                              in_=h_sb[s][lane][0:ms, al:al + CC])

        def phase_c_depth(y_d, s, ms, lane, al):
            a = lane * HALF + al
            x = chunkp.tile([128, CC + 24], f16, tag="X", name="X")
            h0t = chunkp.tile([128, CC], f16, tag="H0", name="H0")
            tt_ = chunkp.tile([128, CC], f16, tag="T", name="T")
            nc.scalar.dma_start(out=x[0:ms], in_=y_d[s][0:ms, a:a + CC + 24])
            nc.scalar.dma_start(out=h0t[0:ms], in_=h0_d[s][0:ms, a:a + CC])
            nc.vector.tensor_scalar(out=tt_[0:ms], in0=x[0:ms, 0:CC],
                                    scalar1=sc_sb[0:ms, s:s + 1],
                                    scalar2=bi_sb[0:ms, s:s + 1],
                                    op0=MUL, op1=ADD)
            nc.vector.tensor_scalar(out=tt_[0:ms, 0:CC:A], in0=x[0:ms, A:CC + A:A],
                                    scalar1=sc_sb[0:ms, s:s + 1],
                                    scalar2=bi_sb[0:ms, s:s + 1],
                                    op0=MUL, op1=ADD)
            nc.vector.tensor_tensor(tt_[0:ms], tt_[0:ms], h0t[0:ms], ADD)
            nc.vector.tensor_scalar_max(h_sb[s][lane][0:ms, al:al + CC], tt_[0:ms], 0.0)

        def phase_c_readout(y_d, s, ms, al):
            x0 = chunkp.tile([128, CC], f16, tag="X0", name="X0")
            tt_ = chunkp.tile([128, CC], f16, tag="T", name="T")
            nc.scalar.dma_start(out=x0[0:ms], in_=y_d[s][0:ms, 1 + al:1 + al + CC])
            nc.vector.tensor_scalar(out=tt_[0:ms], in0=x0[0:ms],
                                    scalar1=sc_sb[0:ms, s:s + 1],
                                    scalar2=bi_sb[0:ms, s:s + 1],
                                    op0=MUL, op1=ADD)
            nc.vector.tensor_scalar_max(h_sb[s][0][0:ms, al:al + CC], tt_[0:ms], 0.0)

        # ================= pipeline body =================
        def l0_chunks(t):
            lane, tt = divmod(t, NTL)
            c0 = tt * CT
            src = af_T if lane == 0 else af_Tr
            a0 = stream.tile([128, CT], f16, tag="a0", name="a0")
            a1 = stream.tile([5, CT], f16, tag="a1", name="a1")
            bt = stream.tile([BOND_DIM, CT], f16, tag="bt", name="bt")
            nc.sync.dma_start(out=a0, in_=src[0:128, c0:c0 + CT])
            nc.sync.dma_start(out=a1, in_=src[128:133, c0:c0 + CT])
            nc.sync.dma_start(out=bt, in_=bf_T[:, t * CT:(t + 1) * CT])
            return [(wi0_sb, a0[:]), (wi1a_sb, a1[:]), (wi1b_sb, bt[:])]

        def new_stats(nt, tact):
            stats6 = statsp.tile([128, 3, max(nt - tact, 1), 6], f32, tag="stats",
                                 name="stats")
            sum_p = statsp.tile([128, 3, max(tact, 1)], f32, tag="sum_p",
                                name="sum_p")
            ssq_p = statsp.tile([128, 3, max(tact, 1)], f32, tag="ssq_p",
                                name="ssq_p")
            return stats6, sum_p, ssq_p

        def body():
            nt = 2 * NTL
            st0 = new_stats(nt, TACT)
            phase_a(y_d2[0], nt, l0_chunks, *st0, TACT)
            phase_b(*st0, nt, TACT, "g_i", "b_i")
            for lane in range(2):
                for i in range(NCHL):
                    for s, (s0, s1) in enumerate(SLABS):
                        phase_c_l0(y_d2[0], s, s1 - s0, lane, i * CC)

            for _d in range(DEPTH):
                par = (_d + 1) % 2

                def d_chunks(t, _d=_d):
                    lane, tt = divmod(t, NTL)
                    return [(wm_sb[i],
                             h_sb[i][lane][0:k1 - k0, tt * CT:(tt + 1) * CT])
                            for i, (k0, k1) in enumerate(KCH)]

                std = new_stats(nt, TACT)
                phase_a(y_d2[par], nt, d_chunks, *std, TACT)
                phase_b(*std, nt, TACT, "g_m", "b_m")
                for lane in range(2):
                    for i in range(NCHL):
                        for s, (s0, s1) in enumerate(SLABS):
                            phase_c_depth(y_d2[par], s, s1 - s0, lane, i * CC)

            # m_v[atom] = h_e[atom] + h_o[reversed-within-mol] ; into e tile
            for s, (s0, s1) in enumerate(SLABS):
                ms = s1 - s0
                he = h_sb[s][0][0:ms, :].rearrange("p (m a) -> p m a", a=A)
                ho = h_sb[s][1][0:ms, :].rearrange("p (m a) -> p m a", a=A)
                ho_rev = bass.AP(
                    tensor=ho.tensor,
                    offset=ho.offset + (A - 1) * ho.ap[2][0],
                    ap=[list(ho.ap[0]), list(ho.ap[1]), [-ho.ap[2][0], A]],
                )
                nc.vector.tensor_tensor(he, he, ho_rev, ADD)

            def ro_chunks(t):
                c0 = t * CT
                a0 = stream.tile([128, CT], f16, tag="a0", name="a0")
                a1 = stream.tile([5, CT], f16, tag="a1", name="a1")
                nc.sync.dma_start(out=a0, in_=af_T[0:128, c0:c0 + CT])
                nc.sync.dma_start(out=a1, in_=af_T[128:133, c0:c0 + CT])
                out = [(wa_af0, a0[:]), (wa_af1, a1[:])]
                for i, (k0, k1) in enumerate(KCH):
                    out.append((wa_mv[i], h_sb[i][0][0:k1 - k0, c0:c0 + CT]))
                return out

            par_r = (DEPTH + 1) % 2
            str_ = new_stats(NTL, TACT_R)
            phase_a(y_d2[par_r], NTL, ro_chunks, *str_, TACT_R)
            phase_b(*str_, NTL, TACT_R, "g_a", "b_a", cnt_g=float(HALF * NCORES))
            for i in range(NCHL):
                for s, (s0, s1) in enumerate(SLABS):
                    phase_c_readout(y_d2[par_r], s, s1 - s0, i * CC)

            # molecule mean (sum here; /24 on host) and output
            for s, (s0, s1) in enumerate(SLABS):
                ms = s1 - s0
                red = statsp.tile([128, mpc], f32, tag="red", name="red")
                nc.vector.tensor_reduce(
                    out=red[0:ms],
                    in_=h_sb[s][0][0:ms, :].rearrange("p (m a) -> p m a", a=A),
                    axis=mybir.AxisListType.X, op=ADD)
                nc.sync.dma_start(out=h_mol[s, 0:ms, :], in_=red[0:ms])

        for _r in range(repeat):
            if _r:
                tc.strict_bb_all_engine_barrier()
            body()

        ctx.close()

    nc.compile()
    return nc


_PROG_CACHE = {}


def _get_program(mpc, repeat=1):
    key = (mpc, repeat)
    if key not in _PROG_CACHE:
        _PROG_CACHE[key] = _build_program(mpc, repeat)
    return _PROG_CACHE[key]


def _assemble_output(inputs, results, mpc):
    mf = np.asarray(inputs["molecule_features"], np.float32)
    outs = []
    for p in range(NCORES):
        hm = results[p]["h_mol"]  # [3, 128, mpc]
        full = np.concatenate([hm[0][0:128], hm[1][0:128], hm[2][0:44]], axis=0)
        hmol = (full.T / float(A)).astype(np.float32)          # [mpc, 300]
        outs.append(np.concatenate([hmol, mf[p * mpc:(p + 1) * mpc]], axis=1))
    return np.concatenate(outs, axis=0)


def kernel(**inputs):
    from concourse.bass_utils import run_bass_kernel_spmd

    mpc = M_TOT // NCORES
    nc = _get_program(mpc)
    in_maps = [_prep_core_inputs(inputs, p, mpc) for p in range(NCORES)]
    res = run_bass_kernel_spmd(nc, in_maps, core_ids=list(range(NCORES)))
    return _assemble_output(inputs, res.results, mpc)


# ---- helpers used by test harness (not by the grader) ----
def run_sim(inputs, m_tot):
    """Simulate the SPMD program on CoreSim (small sizes only)."""
    import concourse.bass_interp as bass_interp
    mpc = m_tot // NCORES
    nc = _build_program(mpc)
    in_maps = [_prep_core_inputs(inputs, p, mpc) for p in range(NCORES)]
    # pads of y_d are read-but-never-consumed; sim fills uninit DRAM with NaN
    sim = bass_interp.MultiCoreSim(nc, NCORES, require_nnan=False,
                                   require_finite=False)
    for p in range(NCORES):
        for k, v in in_maps[p].items():
            sim.cores[p].tensor(k)[:] = v
    sim.simulate(check_with_hw=False)
    results = [{"h_mol": np.array(sim.cores[p].tensor("h_mol"))}
               for p in range(NCORES)]
    # emulate M_TOT override in assembly
    global M_TOT
    return _assemble_output(inputs, results, mpc)



# revision 7
# speedup vs baseline: 1.5216x; 1.5216x over previous
"""Trainium2 Bass kernel for DMPNN encoder (nn_DMPNNEncoder_61907658604848).

Strategy (data-parallel over molecules, 8 NeuronCores):
  - Each molecule is a 24-atom ring => 48 directed bonds. After a host-side
    bond reordering ("eo order": even bonds in atom order, odd bonds reversed
    within each molecule), the message-passing gather
        m = padded[bibm].sum(1) - h[rev]
    becomes a uniform cyclic shift-down-by-1 within every 24-column block.
  - All activations live transposed [channel, bond-col] so W-matmuls contract
    over channels (PE partitions), BN stats are per-partition free-dim
    reductions (bn_stats/bn_aggr + AllReduce of per-core sums/sumsq), and the
    BN affine+relu are per-partition scalar ops.
  - v2: h AND y share the same SBUF residency: phase A evacuates each psum
    column-tile back into the h buffer it just consumed (the tile is dead as
    a matmul rhs once all 3 output slabs contracted it). Phase C transforms
    y -> h fully in SBUF with in-place shifted reads (descending chunk order
    keeps the shift's cross-chunk read ahead of the overwrite). Only h0
    round-trips through DRAM ([128, 3, EC] layout => one DMA per chunk).
  - Matmuls run in groups of TPC column tiles per weight chunk so the PE
    streams back-to-back and weight loads amortize.
"""

import os

os.environ.setdefault("MYCRO_LOCAL_CACHE", "1")

import numpy as np

# ---------------- problem constants (hardcoded per spec) ----------------
M_TOT = 4096
A = 24
H = 300
ATOM_DIM = 133
BOND_DIM = 14
MOL_DIM = 200
DEPTH = 4
EPS = 1e-5
NCORES = 8

SLABS = [(0, 128), (128, 256), (256, 300)]  # output-channel slabs
KCH = [(0, 128), (128, 256), (256, 300)]    # contraction chunks over H

F16 = np.float16


# ---------------- host-side data prep ----------------
def _eo_perm(n_mols):
    """perm[eo_col] = original directed-bond index within the shard."""
    mu = np.arange(n_mols)[:, None]
    v = np.arange(A)[None, :]
    e = (mu * 48 + 2 * v).reshape(-1)                    # e-lane: col mu*24+v
    o = (mu * 48 + 2 * (A - 1 - v) + 1).reshape(-1)      # o-lane col mu*24+w -> v=23-w
    return np.concatenate([e, o])


def _prep_core_inputs(inputs, p, mpc):
    half = mpc * A
    a0 = p * half
    e0 = p * 2 * half
    af = np.asarray(inputs["atom_features"], np.float32)[a0:a0 + half]
    bf = np.asarray(inputs["bond_features"], np.float32)[e0:e0 + 2 * half]
    perm = _eo_perm(mpc)

    af_T = np.ascontiguousarray(af.T, dtype=F16)                      # [133, half]
    af_Tr = np.ascontiguousarray(
        af_T.reshape(ATOM_DIM, mpc, A)[:, :, ::-1].reshape(ATOM_DIM, half))
    bf_T = np.ascontiguousarray(bf[perm].T, dtype=F16)                # [14, 2*half]
    # stacked [af rows 128:133 ; bf rows] per lane: one rhs tile for WiT1
    abf_T = np.concatenate([
        np.concatenate([af_T[128:133], bf_T[:, 0:half]], axis=0),
        np.concatenate([af_Tr[128:133], bf_T[:, half:]], axis=0),
    ], axis=1)                                                        # [19, 2*half]

    W_i = np.asarray(inputs["W_i"], np.float32)   # [300, 147] = [bond14 | atom133]
    W_m = np.asarray(inputs["W_m"], np.float32)
    W_a = np.asarray(inputs["W_a"], np.float32)   # [300, 433] = [atom133 | mv300]
    Wb = W_i[:, :BOND_DIM]
    Wat = W_i[:, BOND_DIM:]
    WiT0 = np.ascontiguousarray(Wat[:, 0:128].T, dtype=F16)           # [128, 300]
    WiT1 = np.ascontiguousarray(
        np.concatenate([Wat[:, 128:133], Wb], axis=1).T, dtype=F16)   # [19, 300]
    WmT = np.ascontiguousarray(W_m.T, dtype=F16)                      # [300, 300]
    WaT = np.ascontiguousarray(W_a.T, dtype=F16)                      # [433, 300]

    d = {
        "af_T": af_T, "af_Tr": af_Tr, "abf_T": np.ascontiguousarray(abf_T, F16),
        "WiT0": WiT0, "WiT1": WiT1, "WmT": WmT, "WaT": WaT,
    }
    for n in ("g_i", "b_i", "g_m", "b_m", "g_a", "b_a"):
        v = np.asarray(inputs[n], np.float32)
        pk = np.zeros((128, 3), np.float32)
        for s, (s0, s1) in enumerate(SLABS):
            pk[0:s1 - s0, s] = v[s0:s1]
        d[n] = pk
    return d


# ---------------- bass program ----------------
def _build_program(mpc, repeat=1):
    import concourse.bass as bass
    import concourse.bacc as bacc
    import concourse.tile as tile
    import concourse.mybir as mybir

    f16 = mybir.dt.float16
    f32 = mybir.dt.float32
    MUL = mybir.AluOpType.mult
    ADD = mybir.AluOpType.add
    SUB = mybir.AluOpType.subtract
    RELU = mybir.ActivationFunctionType.Relu
    COPY = mybir.ActivationFunctionType.Copy

    HALF = mpc * A
    EC = 2 * HALF
    CT = 512 if HALF % 512 == 0 else HALF          # psum col-tile
    assert CT <= 512 and HALF % CT == 0
    CC = 1536 if HALF % 1536 == 0 else HALF        # chunk (phase C / stats / group)
    assert HALF % CC == 0 and CC % A == 0 and CC % CT == 0
    NT = HALF // CT                                # col tiles per lane
    NCHL = HALF // CC                              # chunks per lane
    TPC = CC // CT                                 # tiles per chunk/group
    NB = CC // A                                   # 24-blocks per chunk
    CNT_G = float(EC * NCORES)                     # global batch size (bonds)

    nc = bacc.Bacc("TRN2", target_bir_lowering=False, debug=False,
                   num_devices=NCORES)

    def din(name, shape, dt=f16):
        return nc.dram_tensor(name, shape, dt, kind="ExternalInput").ap()

    af_T = din("af_T", [ATOM_DIM, HALF])
    af_Tr = din("af_Tr", [ATOM_DIM, HALF])
    abf_T = din("abf_T", [19, EC])
    WiT0_d = din("WiT0", [128, H])
    WiT1_d = din("WiT1", [19, H])
    WmT_d = din("WmT", [H, H])
    WaT_d = din("WaT", [433, H])
    gb_d = {n: din(n, [128, 3], f32) for n in ("g_i", "b_i", "g_m", "b_m", "g_a", "b_a")}
    h_mol = nc.dram_tensor("h_mol", [3, 128, mpc], f32, kind="ExternalOutput").ap()

    with tile.TileContext(nc) as tc:
        import contextlib
        ctx = contextlib.ExitStack()

        res = ctx.enter_context(tc.tile_pool(name="res", bufs=1))
        dram = ctx.enter_context(tc.tile_pool(name="dram", bufs=1, space="DRAM"))
        dramx = ctx.enter_context(tc.tile_pool(name="dramx", bufs=2, space="DRAM"))
        psum = ctx.enter_context(tc.tile_pool(name="psum", bufs=8, space="PSUM"))
        stream = ctx.enter_context(tc.tile_pool(name="stream", bufs=2))
        h0p = ctx.enter_context(tc.tile_pool(name="h0p", bufs=2))
        ttp = ctx.enter_context(tc.tile_pool(name="ttp", bufs=2))
        statsp = ctx.enter_context(tc.tile_pool(name="statsp", bufs=2))

        # ---- resident SBUF ----
        hy = [[res.tile([128, HALF], f16, tag=f"h{s}{l}", name=f"h{s}{l}")
               for l in range(2)] for s in range(3)]
        wi0_sb = res.tile([128, H], f16, tag="wi0", name="wi0")
        wi1_sb = res.tile([19, H], f16, tag="wi1", name="wi1")
        wm_sb = [res.tile([k1 - k0, H], f16, tag=f"wm{i}", name=f"wm{i}")
                 for i, (k0, k1) in enumerate(KCH)]
        wa_af0 = res.tile([128, H], f16, tag="wa_af0", name="wa_af0")
        wa_af1 = res.tile([5, H], f16, tag="wa_af1", name="wa_af1")
        wa_mv = [res.tile([k1 - k0, H], f16, tag=f"wa_mv{i}", name=f"wa_mv{i}")
                 for i, (k0, k1) in enumerate(KCH)]
        gb_sb = {n: res.tile([128, 3], f32, tag=f"gb_{n}", name=f"gb_{n}") for n in gb_d}
        eps_sb = res.tile([128, 1], f32, tag="eps", name="eps")
        sc_sb = res.tile([128, 3], f32, tag="sc", name="sc")
        bi_sb = res.tile([128, 3], f32, tag="bi", name="bi")

        nc.vector.memset(eps_sb, EPS)
        nc.sync.dma_start(out=wi0_sb, in_=WiT0_d)
        nc.sync.dma_start(out=wi1_sb, in_=WiT1_d)
        for i, (k0, k1) in enumerate(KCH):
            nc.sync.dma_start(out=wm_sb[i], in_=WmT_d[k0:k1])
            nc.sync.dma_start(out=wa_mv[i], in_=WaT_d[133 + k0:133 + k1])
        nc.sync.dma_start(out=wa_af0, in_=WaT_d[0:128])
        nc.sync.dma_start(out=wa_af1, in_=WaT_d[128:133])
        for n, dap in gb_d.items():
            nc.sync.dma_start(out=gb_sb[n], in_=dap)

        # persistent DRAM scratch for h0, slab-interleaved: [part, slab, bond-col]
        h0_d = dram.tile([128, 3, EC], f16, tag="h0", name="h0")

        # -------- phase A: grouped matmuls -> psum -> f16 evac into hy + stats
        # order within a group: s-outer, k-inner, tile-innermost => TPC
        # back-to-back matmuls per weight chunk; evacs+stats trail the group.
        def phase_a(groups, chunks_fn, dst_fn, stats6):
            for lane, ci in groups:
                tiles = [ci * TPC + q for q in range(TPC)]
                chunks = chunks_fn(lane, ci)   # [tile][k] -> (lhsT, rhs)
                nk = len(chunks[0])
                pst = [[psum.tile([128, CT], f32, tag="ps", name="ps")
                        for _ in range(TPC)] for _ in range(3)]
                for s, (s0, s1) in enumerate(SLABS):
                    ms = s1 - s0
                    for k in range(nk):
                        for q in range(TPC):
                            lh, rh = chunks[q][k]
                            nc.tensor.matmul(pst[s][q][0:ms], lh[:, s0:s1], rh,
                                             start=(k == 0), stop=(k == nk - 1))
                for s, (s0, s1) in enumerate(SLABS):
                    ms = s1 - s0
                    for q in range(TPC):
                        nc.scalar.copy(out=dst_fn(s, lane, tiles[q])[0:ms],
                                       in_=pst[s][q][0:ms])
                for s, (s0, s1) in enumerate(SLABS):
                    ms = s1 - s0
                    for q in range(TPC):
                        t = tiles[q]
                        nc.vector.bn_stats(
                            out=stats6[0:ms, s, lane * NT + t],
                            in_=dst_fn(s, lane, t)[0:ms])

        def dst_hy(s, lane, t):
            return hy[s][lane][:, t * CT:(t + 1) * CT]

        def dst_ro(s, lane, t):
            return hy[s][1][:, t * CT:(t + 1) * CT]

        # -------- phase B: stats -> allreduce -> scale/bias --------
        def phase_b(stats6, nst, gname, bname, cnt_l, cnt_g):
            pack = statsp.tile([128, 6], f32, tag="pack", name="pack")
            micro = statsp.tile([128, 24], f32, tag="micro", name="micro")
            nc.vector.memset(pack, 0.0)
            for s, (s0, s1) in enumerate(SLABS):
                ms = s1 - s0
                mv = micro[0:ms, 8 * s:8 * s + 2]
                nc.vector.bn_aggr(out=mv, in_=stats6[0:ms, s, 0:nst])
                mean = micro[0:ms, 8 * s:8 * s + 1]
                var = micro[0:ms, 8 * s + 1:8 * s + 2]
                sq = micro[0:ms, 8 * s + 2:8 * s + 3]
                ss = micro[0:ms, 8 * s + 3:8 * s + 4]
                # sum = mean*cnt_l ; sumsq = (var + mean^2)*cnt_l
                nc.vector.tensor_tensor(sq, mean, mean, MUL)
                nc.vector.tensor_tensor(ss, var, sq, ADD)
                nc.vector.tensor_scalar_mul(pack[0:ms, 2 * s + 1:2 * s + 2], ss, cnt_l)
                nc.vector.tensor_scalar_mul(pack[0:ms, 2 * s:2 * s + 1], mean, cnt_l)
            din_t = dramx.tile([128, 6], f32, tag="cc_in", name="cc_in")
            dout_t = dramx.tile([128, 6], f32, tag="cc_out", name="cc_out")
            nc.sync.dma_start(out=din_t, in_=pack)
            nc.gpsimd.collective_compute(
                "AllReduce", ADD, replica_groups=[list(range(NCORES))],
                ins=[din_t[:].opt()], outs=[dout_t[:].opt()])
            allst = statsp.tile([128, 6], f32, tag="allst", name="allst")
            nc.sync.dma_start(out=allst, in_=dout_t)
            for s, (s0, s1) in enumerate(SLABS):
                ms = s1 - s0
                mu = micro[0:ms, 8 * s + 4:8 * s + 5]
                ey2 = micro[0:ms, 8 * s + 5:8 * s + 6]
                var = micro[0:ms, 8 * s + 6:8 * s + 7]
                tmp = micro[0:ms, 8 * s + 7:8 * s + 8]
                nc.vector.tensor_scalar_mul(mu, allst[0:ms, 2 * s:2 * s + 1], 1.0 / cnt_g)
                nc.vector.tensor_scalar_mul(ey2, allst[0:ms, 2 * s + 1:2 * s + 2], 1.0 / cnt_g)
                nc.vector.tensor_tensor(tmp, mu, mu, MUL)
                nc.vector.tensor_tensor(var, ey2, tmp, SUB)
                nc.scalar.activation(out=var, in_=var,
                                     func=mybir.ActivationFunctionType.Sqrt,
                                     bias=eps_sb[0:ms, 0:1], scale=1.0)
                nc.vector.reciprocal(out=var, in_=var)
                nc.vector.tensor_tensor(sc_sb[0:ms, s:s + 1],
                                        gb_sb[gname][0:ms, s:s + 1], var, MUL)
                nc.vector.tensor_tensor(tmp, mu, sc_sb[0:ms, s:s + 1], MUL)
                nc.vector.tensor_tensor(bi_sb[0:ms, s:s + 1],
                                        gb_sb[bname][0:ms, s:s + 1], tmp, SUB)

        def new_stats(nst):
            return statsp.tile([128, 3, nst, 6], f32, tag="stats", name="stats")

        # group orders: descending so the next stage's phase A (also
        # descending) can chase this stage's phase C chunk by chunk.
        GROUPS = [(l, ci) for l in (1, 0) for ci in range(NCHL - 1, -1, -1)]
        GROUPS_L0 = [(l, ci) for l in (1, 0) for ci in range(NCHL - 1, -1, -1)]
        GROUPS_RO = [(0, ci) for ci in range(NCHL - 1, -1, -1)]

        # ================= pipeline body =================
        def body():
            # ---------- L0 ----------
            def l0_chunks(lane, ci):
                src = af_T if lane == 0 else af_Tr
                c0 = ci * CC
                e0 = lane * HALF + c0
                a0c = stream.tile([128, CC], f16, tag="a0c", name="a0c")
                abfc = stream.tile([19, CC], f16, tag="abfc", name="abfc")
                nc.sync.dma_start(out=a0c, in_=src[0:128, c0:c0 + CC])
                nc.sync.dma_start(out=abfc, in_=abf_T[:, e0:e0 + CC])
                return [[(wi0_sb, a0c[:, q * CT:(q + 1) * CT]),
                         (wi1_sb, abfc[:, q * CT:(q + 1) * CT])]
                        for q in range(TPC)]

            st0 = new_stats(2 * NT)
            phase_a(GROUPS_L0, l0_chunks, dst_hy, st0)
            phase_b(st0, 2 * NT, "g_i", "b_i", float(EC), CNT_G)
            # phase C L0: h0 = relu(sc*y + bi), in place, one ACT op per chunk
            for lane in (1, 0):
                for ci in range(NCHL - 1, -1, -1):
                    al = ci * CC
                    for s, (s0, s1) in enumerate(SLABS):
                        ms = s1 - s0
                        reg = hy[s][lane][0:ms, al:al + CC]
                        nc.scalar.activation(out=reg, in_=reg, func=RELU,
                                             bias=bi_sb[0:ms, s:s + 1],
                                             scale=sc_sb[0:ms, s:s + 1])
                # h0 -> DRAM (one big DMA per slab+lane)
                for s, (s0, s1) in enumerate(SLABS):
                    ms = s1 - s0
                    nc.sync.dma_start(
                        out=h0_d[0:ms, s, lane * HALF:(lane + 1) * HALF],
                        in_=hy[s][lane][0:ms, :])

            # ---------- depth stages ----------
            for _d in range(DEPTH):
                def d_chunks(lane, ci, _d=_d):
                    out = []
                    for q in range(TPC):
                        c0 = ci * CC + q * CT
                        out.append([
                            (wm_sb[i], hy[i][lane][0:k1 - k0, c0:c0 + CT])
                            for i, (k0, k1) in enumerate(KCH)])
                    return out

                std = new_stats(2 * NT)
                phase_a(GROUPS, d_chunks, dst_hy, std)
                phase_b(std, 2 * NT, "g_m", "b_m", float(EC), CNT_G)
                # phase C depth: h = relu(h0 + sc*y_shifted + bi), in SBUF
                for lane in (1, 0):
                    for ci in range(NCHL - 1, -1, -1):
                        al = ci * CC
                        a = lane * HALF + al
                        h0t = h0p.tile([128, 3, CC], f16, tag="h0t", name="h0t")
                        nc.sync.dma_start(out=h0t, in_=h0_d[0:128, 0:3, a:a + CC])
                        for s, (s0, s1) in enumerate(SLABS):
                            ms = s1 - s0
                            tt = ttp.tile([128, CC], f16, tag="tt", name="tt")
                            ysl = hy[s][lane]
                            if al > 0:
                                nc.vector.tensor_scalar(
                                    out=tt[0:ms], in0=ysl[0:ms, al - 1:al - 1 + CC],
                                    scalar1=sc_sb[0:ms, s:s + 1],
                                    scalar2=bi_sb[0:ms, s:s + 1],
                                    op0=MUL, op1=ADD)
                            else:
                                nc.vector.tensor_scalar(
                                    out=tt[0:ms, 1:CC], in0=ysl[0:ms, 0:CC - 1],
                                    scalar1=sc_sb[0:ms, s:s + 1],
                                    scalar2=bi_sb[0:ms, s:s + 1],
                                    op0=MUL, op1=ADD)
                            # block-start cols wrap: m[kA] = y[kA + 23]
                            nc.vector.tensor_scalar(
                                out=tt[0:ms, 0:CC:A],
                                in0=ysl[0:ms, al + A - 1:al + CC:A],
                                scalar1=sc_sb[0:ms, s:s + 1],
                                scalar2=bi_sb[0:ms, s:s + 1],
                                op0=MUL, op1=ADD)
                            nc.vector.tensor_tensor(tt[0:ms], tt[0:ms],
                                                    h0t[0:ms, s], ADD)
                            nc.scalar.activation(out=ysl[0:ms, al:al + CC],
                                                 in_=tt[0:ms], func=RELU)

            # ---------- readout ----------
            # m_v = h_e + h_o_reversed, chunked descending (into e-lane)
            for ci in range(NCHL - 1, -1, -1):
                al = ci * CC
                for s, (s0, s1) in enumerate(SLABS):
                    ms = s1 - s0
                    he = hy[s][0][0:ms, al:al + CC].rearrange(
                        "p (m a) -> p m a", a=A)
                    ho = hy[s][1][0:ms, al:al + CC].rearrange(
                        "p (m a) -> p m a", a=A)
                    ho_rev = bass.AP(
                        tensor=ho.tensor,
                        offset=ho.offset + (A - 1) * ho.ap[2][0],
                        ap=[list(ho.ap[0]), list(ho.ap[1]), [-ho.ap[2][0], A]],
                    )
                    nc.vector.tensor_tensor(he, he, ho_rev, ADD)

            def ro_chunks(lane, ci):
                c0 = ci * CC
                a0c = stream.tile([128, CC], f16, tag="a0c", name="a0c")
                a1c = stream.tile([5, CC], f16, tag="a1c", name="a1c")
                nc.sync.dma_start(out=a0c, in_=af_T[0:128, c0:c0 + CC])
                nc.sync.dma_start(out=a1c, in_=af_T[128:133, c0:c0 + CC])
                out = []
                for q in range(TPC):
                    cq = c0 + q * CT
                    ch = [(wa_af0, a0c[:, q * CT:(q + 1) * CT]),
                          (wa_af1, a1c[:, q * CT:(q + 1) * CT])]
                    for i, (k0, k1) in enumerate(KCH):
                        ch.append((wa_mv[i], hy[i][0][0:k1 - k0, cq:cq + CT]))
                    out.append(ch)
                return out

            str_ = new_stats(NT)
            phase_a(GROUPS_RO, ro_chunks, dst_ro, str_)
            phase_b(str_, NT, "g_a", "b_a", float(HALF), float(HALF * NCORES))
            for ci in range(NCHL - 1, -1, -1):
                al = ci * CC
                for s, (s0, s1) in enumerate(SLABS):
                    ms = s1 - s0
                    reg = hy[s][1][0:ms, al:al + CC]
                    nc.scalar.activation(out=reg, in_=reg, func=RELU,
                                         bias=bi_sb[0:ms, s:s + 1],
                                         scale=sc_sb[0:ms, s:s + 1])

            # molecule mean (sum here; /24 on host) and output
            for s, (s0, s1) in enumerate(SLABS):
                ms = s1 - s0
                red = statsp.tile([128, mpc], f32, tag="red", name="red")
                nc.vector.tensor_reduce(
                    out=red[0:ms],
                    in_=hy[s][1][0:ms, :].rearrange("p (m a) -> p m a", a=A),
                    axis=mybir.AxisListType.X, op=ADD)
                nc.sync.dma_start(out=h_mol[s, 0:ms, :], in_=red[0:ms])

        for _r in range(repeat):
            if _r:
                tc.strict_bb_all_engine_barrier()
            body()

        ctx.close()

    nc.compile()
    return nc


_PROG_CACHE = {}


def _get_program(mpc, repeat=1):
    key = (mpc, repeat)
    if key not in _PROG_CACHE:
        _PROG_CACHE[key] = _build_program(mpc, repeat)
    return _PROG_CACHE[key]


def _assemble_output(inputs, results, mpc):
    mf = np.asarray(inputs["molecule_features"], np.float32)
    outs = []
    for p in range(NCORES):
        hm = results[p]["h_mol"]  # [3, 128, mpc]
        full = np.concatenate([hm[0][0:128], hm[1][0:128], hm[2][0:44]], axis=0)
        hmol = (full.T / float(A)).astype(np.float32)          # [mpc, 300]
        outs.append(np.concatenate([hmol, mf[p * mpc:(p + 1) * mpc]], axis=1))
    return np.concatenate(outs, axis=0)


def kernel(**inputs):
    from concourse.bass_utils import run_bass_kernel_spmd

    mpc = M_TOT // NCORES
    nc = _get_program(mpc)
    in_maps = [_prep_core_inputs(inputs, p, mpc) for p in range(NCORES)]
    res = run_bass_kernel_spmd(nc, in_maps, core_ids=list(range(NCORES)))
    return _assemble_output(inputs, res.results, mpc)


# ---- helpers used by test harness (not by the grader) ----
def run_sim(inputs, m_tot):
    """Simulate the SPMD program on CoreSim (small sizes only)."""
    import concourse.bass_interp as bass_interp
    mpc = m_tot // NCORES
    nc = _build_program(mpc)
    in_maps = [_prep_core_inputs(inputs, p, mpc) for p in range(NCORES)]
    sim = bass_interp.MultiCoreSim(nc, NCORES, require_nnan=False,
                                   require_finite=False)
    for p in range(NCORES):
        for k, v in in_maps[p].items():
            sim.cores[p].tensor(k)[:] = v
    sim.simulate(check_with_hw=False)
    results = [{"h_mol": np.array(sim.cores[p].tensor("h_mol"))}
               for p in range(NCORES)]
    return _assemble_output(inputs, results, mpc)
